# revision 7
# baseline (speedup 1.0000x reference)
"""MinGRU forward on 8 Trainium2 NeuronCores.

Reference computation (per batch b, fp32):
    k       = x @ Wz + bz                 # [T, H]
    z       = sigmoid(k)
    c       = 1 - z = sigmoid(-k)
    htilde  = g(x @ Wh + bh)              # g(a) = a+0.5 if a>=0 else sigmoid(a)
                                          #      = max(a+0.5, sigmoid(a))
    h[0]    = g(h_0)
    h[t]    = c[t-1]*h[t-1] + z[t-1]*htilde[t-1]   (t = 1..T)
    out     = h                           # [T+1, H]

The log-space cumlogsumexp in the reference is exactly this linear
recurrence (all quantities positive, coefficients in (0,1), so the
linear form is numerically stable).

Sharding: data-parallel over batch, one batch per core, weights
replicated. Inside each core: matmuls run with H on the PSUM partition
dim and T on the free dim, which is the layout tensor_tensor_scan needs
to run the recurrence along T at vector speed.
"""

import numpy as np

B, T, D, H = 8, 4096, 1024, 1024
P = 128
TCH = 512                 # time-chunk (= max fp32 matmul free dim)
NTCH = T // TCH
TS = TCH // P             # 128-row subtiles per chunk
KO = D // P               # contraction tiles
MO = H // P               # output-channel tiles

_PROGRAM_CACHE = {}


def _build_program(mm_dtype="f32r"):
    import concourse.bacc as bacc
    import concourse.mybir as mybir
    import concourse.tile as tile
    from concourse.masks import make_identity

    fp32 = mybir.dt.float32
    if mm_dtype == "f32r":
        mm_dt = mybir.dt.float32r
    elif mm_dtype == "fp32":
        mm_dt = mybir.dt.float32
    else:
        raise ValueError(mm_dtype)
    SIG = mybir.ActivationFunctionType.Sigmoid
    MUL = mybir.AluOpType.mult
    ADD = mybir.AluOpType.add
    MAX = mybir.AluOpType.max

    nc = bacc.Bacc("TRN2", target_bir_lowering=False)

    x_ext = nc.declare_dram_parameter("x", [T, D], fp32, isOutput=False)
    h0_ext = nc.declare_dram_parameter("h_0", [H], fp32, isOutput=False)
    wz_ext = nc.declare_dram_parameter("Wz", [D, H], fp32, isOutput=False)
    bz_ext = nc.declare_dram_parameter("bz", [H], fp32, isOutput=False)
    wh_ext = nc.declare_dram_parameter("Wh", [D, H], fp32, isOutput=False)
    bh_ext = nc.declare_dram_parameter("bh", [H], fp32, isOutput=False)
    out_ext = nc.declare_dram_parameter("out", [T + 1, H], fp32, isOutput=True)

    with tile.TileContext(nc) as tc:
        with (
            tc.tile_pool(name="const", bufs=1) as const_pool,
            tc.tile_pool(name="w", bufs=1) as w_pool,
            tc.tile_pool(name="xin", bufs=2) as x_pool,
            tc.tile_pool(name="xt", bufs=2) as xt_pool,
            tc.tile_pool(name="ht", bufs=2) as ht_pool,
            tc.tile_pool(name="gate", bufs=2) as gate_pool,
            tc.tile_pool(name="o", bufs=2) as o_pool,
            tc.tile_pool(name="psx", bufs=2, space="PSUM") as psum_x,
            tc.tile_pool(name="psp", bufs=2, space="PSUM") as psum_p,
            tc.tile_pool(name="pso", bufs=2, space="PSUM") as psum_o,
        ):
            identity = const_pool.tile([P, P], fp32)
            make_identity(nc, identity)

            # Weights resident: [ki, ko, h] so lhsT tiles are natural slices.
            # Tiles carry the matmul dtype; the DMA is a byte copy (both
            # sides bitcast), the PE rounds on read in f32r mode.
            wz_sb = w_pool.tile([P, KO, H], mm_dt)
            nc.sync.dma_start(
                wz_sb, wz_ext.bitcast(mm_dt).rearrange("(ko ki) h -> ki ko h", ki=P)
            )
            wh_sb = w_pool.tile([P, KO, H], mm_dt)
            nc.sync.dma_start(
                wh_sb, wh_ext.bitcast(mm_dt).rearrange("(ko ki) h -> ki ko h", ki=P)
            )

            # Per-channel columns: partition = channel-within-tile, free = tile.
            bz_sb = const_pool.tile([P, MO], fp32)
            nc.sync.dma_start(bz_sb, bz_ext.rearrange("(mo mi) -> mi mo", mi=P))
            nbz_sb = const_pool.tile([P, MO], fp32)
            nc.vector.tensor_scalar_mul(nbz_sb, bz_sb, -1.0)
            bh_sb = const_pool.tile([P, MO], fp32)
            nc.sync.dma_start(bh_sb, bh_ext.rearrange("(mo mi) -> mi mo", mi=P))
            bhp5_sb = const_pool.tile([P, MO], fp32)
            nc.vector.tensor_scalar_add(bhp5_sb, bh_sb, 0.5)

            # h[0] = g(h_0) = max(h_0 + 0.5, sigmoid(h_0))
            h0_sb = const_pool.tile([P, MO], fp32)
            nc.sync.dma_start(h0_sb, h0_ext.rearrange("(mo mi) -> mi mo", mi=P))
            s0_sb = const_pool.tile([P, MO], fp32)
            nc.scalar.activation(s0_sb, h0_sb, SIG)
            gh0_sb = const_pool.tile([P, MO], fp32)
            nc.vector.scalar_tensor_tensor(gh0_sb, h0_sb, 0.5, s0_sb, op0=ADD, op1=MAX)
            nc.sync.dma_start(out_ext[0, :].rearrange("(mo mi) -> mi mo", mi=P), gh0_sb)

            prev_ht = None  # previous chunk's scan output (carries the state)
            for ci in range(NTCH):
                # ---- load x chunk & transpose to [d, t] ----
                x_sb = x_pool.tile([P, TS, D], fp32)
                nc.sync.dma_start(
                    x_sb,
                    x_ext[ci * TCH:(ci + 1) * TCH, :].rearrange(
                        "(ts ti) d -> ti ts d", ti=P
                    ),
                )
                xt_sb = xt_pool.tile([P, KO, TCH], mm_dt)
                for ko in range(KO):
                    psx = psum_x.tile([P, TCH], fp32)
                    for tsub in range(TS):
                        nc.tensor.transpose(
                            psx[:, tsub * P:(tsub + 1) * P],
                            x_sb[:, tsub, ko * P:(ko + 1) * P],
                            identity,
                        )
                    nc.any.tensor_copy(xt_sb[:, ko, :], psx)

                # ---- per h-tile: matmuls, gates, scan ----
                ht_sb = ht_pool.tile([P, MO, TCH], fp32)
                for m in range(MO):
                    pk = psum_p.tile([P, TCH], fp32, tag="pk")
                    pa = psum_p.tile([P, TCH], fp32, tag="pa")
                    for ko in range(KO):
                        nc.tensor.matmul(
                            pk,
                            wz_sb[:, ko, m * P:(m + 1) * P],
                            xt_sb[:, ko, :],
                            start=(ko == 0),
                            stop=(ko == KO - 1),
                        )
                    for ko in range(KO):
                        nc.tensor.matmul(
                            pa,
                            wh_sb[:, ko, m * P:(m + 1) * P],
                            xt_sb[:, ko, :],
                            start=(ko == 0),
                            stop=(ko == KO - 1),
                        )

                    c_sb = gate_pool.tile([P, TCH], fp32, tag="c")
                    nc.scalar.activation(
                        c_sb, pk, SIG, bias=nbz_sb[:, m:m + 1], scale=-1.0
                    )
                    z_sb = gate_pool.tile([P, TCH], fp32, tag="z")
                    nc.scalar.activation(z_sb, pk, SIG, bias=bz_sb[:, m:m + 1])
                    s_sb = gate_pool.tile([P, TCH], fp32, tag="s")
                    nc.scalar.activation(s_sb, pa, SIG, bias=bh_sb[:, m:m + 1])
                    g_sb = gate_pool.tile([P, TCH], fp32, tag="g")
                    nc.vector.scalar_tensor_tensor(
                        g_sb, pa, bhp5_sb[:, m:m + 1], s_sb, op0=ADD, op1=MAX
                    )
                    v_sb = gate_pool.tile([P, TCH], fp32, tag="v")
                    nc.vector.tensor_mul(v_sb, z_sb, g_sb)

                    init = (
                        gh0_sb[:, m:m + 1]
                        if prev_ht is None
                        else prev_ht[:, m, TCH - 1:TCH]
                    )
                    nc.vector.tensor_tensor_scan(
                        ht_sb[:, m, :], c_sb, v_sb, init, op0=MUL, op1=ADD
                    )

                # ---- transpose h back to [t, h] rows and store ----
                GP = min(4, MO)
                for tsub in range(TS):
                    o_sb = o_pool.tile([P, H], fp32)
                    for mh in range(MO // GP):
                        pso = psum_o.tile([P, GP * P], fp32)
                        for mq in range(GP):
                            m = mh * GP + mq
                            nc.tensor.transpose(
                                pso[:, mq * P:(mq + 1) * P],
                                ht_sb[:, m, tsub * P:(tsub + 1) * P],
                                identity,
                            )
                        nc.any.tensor_copy(o_sb[:, mh * GP * P:(mh + 1) * GP * P], pso)
                    row0 = 1 + ci * TCH + tsub * P
                    nc.sync.dma_start(out_ext[row0:row0 + P, :], o_sb)

                prev_ht = ht_sb

    nc.finalize()
    return nc


def _get_program(mm_dtype="f32r"):
    if mm_dtype not in _PROGRAM_CACHE:
        _PROGRAM_CACHE[mm_dtype] = _build_program(mm_dtype)
    return _PROGRAM_CACHE[mm_dtype]


def run(x, h_0, Wz, bz, Wh, bh, mm_dtype="f32r", trace=False):
    from concourse.bass_utils import run_bass_kernel_spmd

    nc = _get_program(mm_dtype)
    in_maps = [
        {
            "x": np.ascontiguousarray(x[b], dtype=np.float32),
            "h_0": np.ascontiguousarray(h_0[b].reshape(H), dtype=np.float32),
            "Wz": np.asarray(Wz, dtype=np.float32),
            "bz": np.asarray(bz, dtype=np.float32),
            "Wh": np.asarray(Wh, dtype=np.float32),
            "bh": np.asarray(bh, dtype=np.float32),
        }
        for b in range(B)
    ]
    res = run_bass_kernel_spmd(nc, in_maps, list(range(B)), trace=trace)
    out = np.stack([res.results[b]["out"] for b in range(B)], axis=0)
    return out, res


def kernel(x, h_0, Wz, bz, Wh, bh):
    out, _ = run(x, h_0, Wz, bz, Wh, bh)
    return out


# revision 8
# speedup vs baseline: 1.3796x; 1.3796x over previous
"""MinGRU forward on 8 Trainium2 NeuronCores.

Reference computation (per batch b):
    k       = x @ Wz + bz                 # [T, H]
    z       = sigmoid(k)
    c       = 1 - z = sigmoid(-k)
    htilde  = g(x @ Wh + bh)              # g(a) = a+0.5 if a>=0 else sigmoid(a)
                                          #      = max(a+0.5, sigmoid(a))
    h[0]    = g(h_0)
    h[t]    = c[t-1]*h[t-1] + z[t-1]*htilde[t-1]   (t = 1..T)
    out     = h                           # [T+1, H]

The log-space cumlogsumexp in the reference is exactly this linear
recurrence (all quantities positive, coefficients in (0,1), so the
linear form is numerically stable).

Sharding: data-parallel over batch, one batch per core, weights
replicated.

Device layout: matmuls run with H on the PSUM partition dim and T on
the free dim — the layout tensor_tensor_scan needs to run the
recurrence along T at vector speed. x and the weights are cast to fp16
on the host (matmul accumulates in fp32; ~2^-11 operand rounding); x^T
tiles are produced by DMA-transpose (2-byte dtype), so the TensorEngine
does nothing but the 1024 matmuls per core. The device writes the
output transposed ([H, T+1]); the host transposes during the unshard.
"""

import numpy as np

B, T, D, H = 8, 4096, 1024, 1024
P = 128
TCH = 512                 # time-chunk (one PSUM bank of fp32 per matmul)
NTCH = T // TCH
KO = D // P               # contraction tiles
MO = H // P               # output-channel tiles

_PROGRAM_CACHE = {}


def _build_program():
    import concourse.bacc as bacc
    import concourse.mybir as mybir
    import concourse.tile as tile

    fp32 = mybir.dt.float32
    fp16 = mybir.dt.float16
    SIG = mybir.ActivationFunctionType.Sigmoid
    MUL = mybir.AluOpType.mult
    ADD = mybir.AluOpType.add
    MAX = mybir.AluOpType.max

    nc = bacc.Bacc("TRN2", target_bir_lowering=False)

    x_ext = nc.declare_dram_parameter("x", [T, D], fp16, isOutput=False)
    h0_ext = nc.declare_dram_parameter("h_0", [H], fp32, isOutput=False)
    wz_ext = nc.declare_dram_parameter("Wz", [D, H], fp16, isOutput=False)
    bz_ext = nc.declare_dram_parameter("bz", [H], fp32, isOutput=False)
    wh_ext = nc.declare_dram_parameter("Wh", [D, H], fp16, isOutput=False)
    bh_ext = nc.declare_dram_parameter("bh", [H], fp32, isOutput=False)
    # transposed output; host untransposes during the gather
    out_ext = nc.declare_dram_parameter("out", [H, T + 1], fp32, isOutput=True)

    with tile.TileContext(nc) as tc:
        with (
            tc.tile_pool(name="const", bufs=1) as const_pool,
            tc.tile_pool(name="w", bufs=1) as w_pool,
            tc.tile_pool(name="xt", bufs=3) as xt_pool,
            tc.tile_pool(name="ht", bufs=2) as ht_pool,
            tc.tile_pool(name="gate", bufs=3) as gate_pool,
            tc.tile_pool(name="psp", bufs=4, space="PSUM") as psum_p,
        ):
            # Weights resident: [ki, ko, h] so lhsT tiles are natural slices.
            wz_sb = w_pool.tile([P, KO, H], fp16)
            nc.sync.dma_start(wz_sb, wz_ext.rearrange("(ko ki) h -> ki ko h", ki=P))
            wh_sb = w_pool.tile([P, KO, H], fp16)
            nc.sync.dma_start(wh_sb, wh_ext.rearrange("(ko ki) h -> ki ko h", ki=P))

            # Per-channel columns: partition = channel-within-tile, free = tile.
            bz_sb = const_pool.tile([P, MO], fp32)
            nc.sync.dma_start(bz_sb, bz_ext.rearrange("(mo mi) -> mi mo", mi=P))
            nbz_sb = const_pool.tile([P, MO], fp32)
            nc.vector.tensor_scalar_mul(nbz_sb, bz_sb, -1.0)
            bh_sb = const_pool.tile([P, MO], fp32)
            nc.sync.dma_start(bh_sb, bh_ext.rearrange("(mo mi) -> mi mo", mi=P))
            bhp5_sb = const_pool.tile([P, MO], fp32)
            nc.vector.tensor_scalar_add(bhp5_sb, bh_sb, 0.5)

            # h[0] = g(h_0) = max(h_0 + 0.5, sigmoid(h_0))
            h0_sb = const_pool.tile([P, MO], fp32)
            nc.sync.dma_start(h0_sb, h0_ext.rearrange("(mo mi) -> mi mo", mi=P))
            s0_sb = const_pool.tile([P, MO], fp32)
            nc.scalar.activation(s0_sb, h0_sb, SIG)
            gh0_sb = const_pool.tile([P, MO], fp32)
            nc.vector.scalar_tensor_tensor(gh0_sb, h0_sb, 0.5, s0_sb, op0=ADD, op1=MAX)
            nc.sync.dma_start(out_ext[:, 0].rearrange("(mo mi) -> mi mo", mi=P), gh0_sb)

            prev_ht = None  # previous chunk's scan output (carries the state)
            for ci in range(NTCH):
                # x^T chunk [d, t] via DMA-transpose (no TensorE involved)
                xt_sb = xt_pool.tile([P, KO, TCH], fp16)
                nc.sync.dma_start_transpose(
                    xt_sb, x_ext[ci * TCH:(ci + 1) * TCH, :]
                )

                ht_sb = ht_pool.tile([P, MO, TCH], fp32)
                for m in range(MO):
                    pk = psum_p.tile([P, TCH], fp32, tag="pk")
                    pa = psum_p.tile([P, TCH], fp32, tag="pa")
                    for ko in range(KO):
                        nc.tensor.matmul(
                            pk,
                            wz_sb[:, ko, m * P:(m + 1) * P],
                            xt_sb[:, ko, :],
                            start=(ko == 0),
                            stop=(ko == KO - 1),
                        )
                    for ko in range(KO):
                        nc.tensor.matmul(
                            pa,
                            wh_sb[:, ko, m * P:(m + 1) * P],
                            xt_sb[:, ko, :],
                            start=(ko == 0),
                            stop=(ko == KO - 1),
                        )

                    c_sb = gate_pool.tile([P, TCH], fp32, tag="c")
                    nc.scalar.activation(
                        c_sb, pk, SIG, bias=nbz_sb[:, m:m + 1], scale=-1.0
                    )
                    z_sb = gate_pool.tile([P, TCH], fp32, tag="z")
                    nc.scalar.activation(z_sb, pk, SIG, bias=bz_sb[:, m:m + 1])
                    s_sb = gate_pool.tile([P, TCH], fp32, tag="s")
                    nc.scalar.activation(s_sb, pa, SIG, bias=bh_sb[:, m:m + 1])
                    g_sb = gate_pool.tile([P, TCH], fp32, tag="g")
                    nc.vector.scalar_tensor_tensor(
                        g_sb, pa, bhp5_sb[:, m:m + 1], s_sb, op0=ADD, op1=MAX
                    )
                    v_sb = gate_pool.tile([P, TCH], fp32, tag="v")
                    nc.gpsimd.tensor_mul(v_sb, z_sb, g_sb)

                    init = (
                        gh0_sb[:, m:m + 1]
                        if prev_ht is None
                        else prev_ht[:, m, TCH - 1:TCH]
                    )
                    nc.vector.tensor_tensor_scan(
                        ht_sb[:, m, :], c_sb, v_sb, init, op0=MUL, op1=ADD
                    )
                    nc.sync.dma_start(
                        out_ext[m * P:(m + 1) * P, 1 + ci * TCH:1 + (ci + 1) * TCH],
                        ht_sb[:, m, :],
                    )

                prev_ht = ht_sb

    nc.finalize()
    return nc


def _get_program():
    if "v2" not in _PROGRAM_CACHE:
        _PROGRAM_CACHE["v2"] = _build_program()
    return _PROGRAM_CACHE["v2"]


def run(x, h_0, Wz, bz, Wh, bh, trace=False):
    from concourse.bass_utils import run_bass_kernel_spmd

    nc = _get_program()
    wz16 = np.ascontiguousarray(np.asarray(Wz, dtype=np.float16))
    wh16 = np.ascontiguousarray(np.asarray(Wh, dtype=np.float16))
    bz32 = np.ascontiguousarray(np.asarray(bz, dtype=np.float32))
    bh32 = np.ascontiguousarray(np.asarray(bh, dtype=np.float32))
    in_maps = [
        {
            "x": np.ascontiguousarray(x[b], dtype=np.float16),
            "h_0": np.ascontiguousarray(
                np.asarray(h_0[b], dtype=np.float32).reshape(H)
            ),
            "Wz": wz16,
            "bz": bz32,
            "Wh": wh16,
            "bh": bh32,
        }
        for b in range(B)
    ]
    res = run_bass_kernel_spmd(nc, in_maps, list(range(B)), trace=trace)
    out = np.stack(
        [np.ascontiguousarray(res.results[b]["out"].T) for b in range(B)], axis=0
    )
    return out, res


def kernel(x, h_0, Wz, bz, Wh, bh):
    out, _ = run(x, h_0, Wz, bz, Wh, bh)
    return out
